# revision 14
# baseline (speedup 1.0000x reference)
"""BagModel (segment_reduce) Trainium2 kernel.

Computes out = (1/64 * segment_sum(relu(x @ W1 + b1))) @ W2 + b2 for
4096 bags of exactly 64 consecutive rows each, sharded bag-aligned
across 8 NeuronCores (512 bags / 32768 rows per core, weights
replicated, no cross-core communication).

Layout trick: the host permutes each core's x-shard to
    xh[p, k, g*512 + b] = x[b*64 + g, 128*k + p]
so row-group g contains row g of every bag, with the contraction dim D
on partitions.  The per-bag segment-sum then falls out of PSUM matmul
accumulation: the second (W2) matmul accumulates over the 64 row-groups
with start=(g==0)/stop=(g==63), so no explicit reduction pass over h is
ever needed.  The 4 H-slices of the W2 matmul go to 4 distinct PE
column-groups (tile_position) so they overlap in the array.
"""

import numpy as np

import concourse.bass as bass
import concourse.tile as tile
from concourse import bacc, mybir

N, D, H, C = 262144, 256, 512, 10
N_BAGS, BAG_SIZE = 4096, 64
N_CORES = 8
R = N // N_CORES            # rows per core
BPC = N_BAGS // N_CORES     # bags per core == free dim of each row-group
KT = D // 128               # contraction tiles (2)
MT = H // 128               # H tiles (4)

F32 = mybir.dt.float32
AF = mybir.ActivationFunctionType
ALU = mybir.AluOpType


def build(nc: bass.Bass, bag: int = BAG_SIZE, bpc: int = BPC):
    """Emit the per-core program.  bag = rows per bag (= number of
    row-groups), bpc = bags per core (= free dim, <= 512)."""
    r = bag * bpc
    xT = nc.declare_dram_parameter("xh", [128, KT, r], F32, isOutput=False)
    w1 = nc.declare_dram_parameter("w1h", [128, KT, H], F32, isOutput=False)
    b1 = nc.declare_dram_parameter("b1h", [128, MT], F32, isOutput=False)
    w2 = nc.declare_dram_parameter("w2h", [128, MT, C], F32, isOutput=False)
    b2 = nc.declare_dram_parameter("b2h", [C, 1], F32, isOutput=False)
    out = nc.declare_dram_parameter("out", [C, bpc], F32, isOutput=True)

    with tile.TileContext(nc) as tc:
        with (
            tc.tile_pool(name="const", bufs=1) as cpool,
            # bufs=8 matches the 8-queue HWDGE rotation: slot reuse then
            # pairs WAW deps on the same queue (implicit FIFO, no extra
            # sync wait — walrus allows only one non-self wait per inst)
            tc.tile_pool(name="xin", bufs=8) as xpool,
            tc.tile_pool(name="hrelu", bufs=8) as hpool,
            tc.tile_pool(name="fin", bufs=1) as fpool,
            tc.tile_pool(name="ps_ht", bufs=4, space="PSUM") as pspool,
            tc.tile_pool(name="ps_out", bufs=1, space="PSUM") as popool,
        ):
            w1_sb = cpool.tile([128, KT, H], F32)
            nc.sync.dma_start(out=w1_sb[:], in_=w1[:])
            b1_sb = cpool.tile([128, MT], F32)
            nc.sync.dma_start(out=b1_sb[:], in_=b1[:])
            w2_sb = cpool.tile([128, MT, C], F32)
            nc.sync.dma_start(out=w2_sb[:], in_=w2[:])
            b2_sb = cpool.tile([C, 1], F32)
            nc.sync.dma_start(out=b2_sb[:], in_=b2[:])

            # bag-sum accumulators: col-group m holds partial (over H slice
            # m) of out.T at partitions [32m, 32m+10) of its own bank —
            # start=True zeroes has_written bits bank-wide, so interleaved
            # accumulation groups must not share a bank
            out_ps = [popool.tile([128, bpc], F32, tag=f"out{m}",
                                  name=f"out_ps{m}")
                      for m in range(MT)]

            # warm-up matmuls: absorb the weight-DMA queue waits into PE's
            # vector clock so steady-state matmuls carry <=1 sync wait
            # (walrus limit); outputs are garbage, overwritten by the
            # start=True of the g==0 accumulation groups below.
            nc.tensor.matmul(
                out_ps[0][:, :], lhsT=w1_sb[:, 0, 0:128],
                rhs=w1_sb[:, 0, 0:bpc], start=True, stop=True,
            )
            nc.tensor.matmul(
                out_ps[1][0:C, :], lhsT=w2_sb[:, 0, :],
                rhs=w1_sb[:, 0, 0:bpc], start=True, stop=True,
            )
            # same for the bias tiles on ACT and DVE (first-touch waits)
            warm_a = cpool.tile([128, 1], F32, name="warm_a")
            nc.scalar.activation(warm_a[:], b1_sb[:, 0:1], AF.Relu,
                                 bias=b1_sb[:, 0:1], scale=1.0)
            warm_b = cpool.tile([C, 1], F32, name="warm_b")
            nc.scalar.activation(warm_b[:], b2_sb[:], AF.Relu,
                                 bias=b2_sb[:], scale=1.0)
            warm_v = cpool.tile([128, 1], F32, name="warm_v")
            nc.vector.tensor_scalar(
                out=warm_v[:], in0=b1_sb[:, 0:1], scalar1=b1_sb[:, 0:1],
                scalar2=0.0, op0=ALU.add, op1=ALU.max,
            )

            for g in range(bag):
                xk = xpool.tile([128, KT, bpc], F32, tag="xk")
                nc.sync.dma_start(
                    out=xk[:], in_=xT[:, :, g * bpc:(g + 1) * bpc]
                )
                for m in range(MT):
                    ht = pspool.tile([128, bpc], F32, tag="ht")
                    for k in range(KT):
                        nc.tensor.matmul(
                            ht[:],
                            lhsT=w1_sb[:, k, 128 * m:128 * (m + 1)],
                            rhs=xk[:, k, :],
                            start=(k == 0),
                            stop=(k == KT - 1),
                        )
                    htr = hpool.tile([128, bpc], F32, tag="htr")
                    if m < 2:
                        nc.scalar.activation(
                            htr[:], ht[:], AF.Relu,
                            bias=b1_sb[:, m:m + 1], scale=1.0,
                        )
                    else:
                        nc.vector.tensor_scalar(
                            out=htr[:], in0=ht[:],
                            scalar1=b1_sb[:, m:m + 1], scalar2=0.0,
                            op0=ALU.add, op1=ALU.max,
                        )
                    nc.tensor.matmul(
                        out_ps[m][32 * m:32 * m + C, :],
                        lhsT=w2_sb[:, m, :],
                        rhs=htr[:],
                        start=(g == 0),
                        stop=(g == bag - 1),
                        tile_position=(0, 32 * m),
                    )

            # DVE can read at most one PSUM operand per op: chain via SBUF
            acc = fpool.tile([C, bpc], F32, tag="acc")
            nc.vector.tensor_copy(acc[:], out_ps[0][0:C, :])
            for m in range(1, MT):
                nc.vector.tensor_add(
                    acc[:], acc[:], out_ps[m][32 * m:32 * m + C, :])
            fin = fpool.tile([C, bpc], F32, tag="fin")
            nc.scalar.add(fin[:], acc[:], add=b2_sb[:])
            nc.sync.dma_start(out=out[:], in_=fin[:])


def host_prep_shared(W1, b1, W2, b2, bag=BAG_SIZE):
    w1h = np.ascontiguousarray(
        W1.reshape(KT, 128, H).transpose(1, 0, 2)).astype(np.float32)
    b1h = np.ascontiguousarray(
        b1.reshape(MT, 128).T).astype(np.float32)
    w2h = np.ascontiguousarray(
        (W2 / bag).reshape(MT, 128, C).transpose(1, 0, 2)).astype(np.float32)
    b2h = np.ascontiguousarray(b2.reshape(C, 1)).astype(np.float32)
    return {"w1h": w1h, "b1h": b1h, "w2h": w2h, "b2h": b2h}


def host_prep_x(xs, bag=BAG_SIZE):
    """xs: [r, D] rows of one core -> xh [128, KT, r] permuted."""
    r = xs.shape[0]
    bpc = r // bag
    xh = xs.reshape(bpc, bag, KT, 128).transpose(3, 2, 1, 0).reshape(128, KT, r)
    return np.ascontiguousarray(xh).astype(np.float32)


_BUILT = None


def _get_built():
    global _BUILT
    if _BUILT is None:
        nc = bacc.Bacc("TRN2")
        build(nc)
        nc.compile()
        _BUILT = nc
    return _BUILT


def run(x, W1, b1, W2, b2, ids=None, trace=False):
    from concourse.bass_utils import run_bass_kernel_spmd

    nc = _get_built()
    shared = host_prep_shared(W1, b1, W2, b2)
    in_maps = []
    for c in range(N_CORES):
        xs = np.asarray(x[c * R:(c + 1) * R])
        in_maps.append({"xh": host_prep_x(xs), **shared})
    res = run_bass_kernel_spmd(
        nc, in_maps, core_ids=list(range(N_CORES)), trace=trace
    )
    outs = [res.results[c]["out"] for c in range(N_CORES)]
    full = np.concatenate([o.T for o in outs], axis=0).astype(np.float32)
    return full, res


def kernel(x, W1, b1, W2, b2, ids=None):
    full, _ = run(x, W1, b1, W2, b2, ids)
    return full


# revision 17
# speedup vs baseline: 3.5182x; 3.5182x over previous
"""BagModel (segment_reduce) Trainium2 kernel.

Computes out = (1/64 * segment_sum(relu(x @ W1 + b1))) @ W2 + b2 for
4096 bags of exactly 64 consecutive rows each, sharded bag-aligned
across 8 NeuronCores (512 bags / 32768 rows per core, weights
replicated, no cross-core communication).

Layout trick: the host permutes each core's x-shard to
    xh[p, k, g*512 + b] = x[b*64 + g, 128*k + p]
so row-group g contains row g of every bag, with the contraction dim D
on partitions.  The per-bag segment-sum then falls out of PSUM matmul
accumulation: the second (W2) matmul accumulates over the 64 row-groups
with start=(g==0)/stop=(g==63), so no explicit reduction pass over h is
ever needed.  The 4 H-slices of the W2 matmul go to 4 distinct PE
column-groups (tile_position) so they overlap in the array.
"""

import numpy as np

import concourse.bass as bass
import concourse.tile as tile
from concourse import bacc, mybir

N, D, H, C = 262144, 256, 512, 10
N_BAGS, BAG_SIZE = 4096, 64
N_CORES = 8
R = N // N_CORES            # rows per core
BPC = N_BAGS // N_CORES     # bags per core == free dim of each row-group
KT = D // 128               # contraction tiles (2)
MT = H // 128               # H tiles (4)

F32 = mybir.dt.float32
BF16 = mybir.dt.bfloat16
AF = mybir.ActivationFunctionType
ALU = mybir.AluOpType

# compute dtype for the matmul operands: bf16 is 4x faster on the PE
# (fp32 lowers to 2 LOW/HIGH passes at 2 cycles/elem); accumulation
# stays fp32 in PSUM either way
CDT = BF16


def build(nc: bass.Bass, bag: int = BAG_SIZE, bpc: int = BPC, cdt=None):
    """Emit the per-core program.  bag = rows per bag (= number of
    row-groups), bpc = bags per core (= free dim, <= 512)."""
    if cdt is None:
        cdt = CDT
    r = bag * bpc
    xT = nc.declare_dram_parameter("xh", [128, KT, r], cdt, isOutput=False)
    w1 = nc.declare_dram_parameter("w1h", [128, KT, H], cdt, isOutput=False)
    b1 = nc.declare_dram_parameter("b1h", [128, MT], F32, isOutput=False)
    w2 = nc.declare_dram_parameter("w2h", [128, MT, C], cdt, isOutput=False)
    b2 = nc.declare_dram_parameter("b2h", [C, 1], F32, isOutput=False)
    out = nc.declare_dram_parameter("out", [C, bpc], F32, isOutput=True)

    with tile.TileContext(nc) as tc:
        with (
            tc.tile_pool(name="const", bufs=1) as cpool,
            # bufs=8 matches the 8-queue HWDGE rotation: slot reuse then
            # pairs WAW deps on the same queue (implicit FIFO, no extra
            # sync wait — walrus allows only one non-self wait per inst)
            tc.tile_pool(name="xin", bufs=8) as xpool,
            tc.tile_pool(name="hrelu", bufs=8) as hpool,
            tc.tile_pool(name="fin", bufs=1) as fpool,
            tc.tile_pool(name="ps_ht", bufs=4, space="PSUM") as pspool,
            tc.tile_pool(name="ps_out", bufs=1, space="PSUM") as popool,
        ):
            w1_sb = cpool.tile([128, KT, H], cdt)
            nc.sync.dma_start(out=w1_sb[:], in_=w1[:])
            b1_sb = cpool.tile([128, MT], F32)
            nc.sync.dma_start(out=b1_sb[:], in_=b1[:])
            w2_sb = cpool.tile([128, MT, C], cdt)
            nc.sync.dma_start(out=w2_sb[:], in_=w2[:])
            b2_sb = cpool.tile([C, 1], F32)
            nc.sync.dma_start(out=b2_sb[:], in_=b2[:])

            # bag-sum accumulators: col-group m holds partial (over H slice
            # m) of out.T at partitions [32m, 32m+10) of its own bank —
            # start=True zeroes has_written bits bank-wide, so interleaved
            # accumulation groups must not share a bank
            out_ps = [popool.tile([128, bpc], F32, tag=f"out{m}",
                                  name=f"out_ps{m}")
                      for m in range(MT)]

            # warm-up matmuls: absorb the weight-DMA queue waits into PE's
            # vector clock so steady-state matmuls carry <=1 sync wait
            # (walrus limit); outputs are garbage, overwritten by the
            # start=True of the g==0 accumulation groups below.
            nc.tensor.matmul(
                out_ps[0][:, :], lhsT=w1_sb[:, 0, 0:128],
                rhs=w1_sb[:, 0, 0:bpc], start=True, stop=True,
            )
            nc.tensor.matmul(
                out_ps[1][0:C, :], lhsT=w2_sb[:, 0, :],
                rhs=w1_sb[:, 0, 0:bpc], start=True, stop=True,
            )
            # same for the bias tiles on ACT and DVE (first-touch waits)
            warm_a = cpool.tile([128, 1], F32, name="warm_a")
            nc.scalar.activation(warm_a[:], b1_sb[:, 0:1], AF.Relu,
                                 bias=b1_sb[:, 0:1], scale=1.0)
            warm_b = cpool.tile([C, 1], F32, name="warm_b")
            nc.scalar.activation(warm_b[:], b2_sb[:], AF.Relu,
                                 bias=b2_sb[:], scale=1.0)
            warm_v = cpool.tile([128, 1], F32, name="warm_v")
            nc.vector.tensor_scalar(
                out=warm_v[:], in0=b1_sb[:, 0:1], scalar1=b1_sb[:, 0:1],
                scalar2=0.0, op0=ALU.add, op1=ALU.max,
            )

            for g in range(bag):
                xk = xpool.tile([128, KT, bpc], cdt, tag="xk")
                nc.sync.dma_start(
                    out=xk[:], in_=xT[:, :, g * bpc:(g + 1) * bpc]
                )
                for m in range(MT):
                    ht = pspool.tile([128, bpc], F32, tag="ht")
                    for k in range(KT):
                        nc.tensor.matmul(
                            ht[:],
                            lhsT=w1_sb[:, k, 128 * m:128 * (m + 1)],
                            rhs=xk[:, k, :],
                            start=(k == 0),
                            stop=(k == KT - 1),
                        )
                    htr = hpool.tile([128, bpc], cdt, tag="htr")
                    if m < 2:
                        nc.scalar.activation(
                            htr[:], ht[:], AF.Relu,
                            bias=b1_sb[:, m:m + 1], scale=1.0,
                        )
                    else:
                        nc.vector.tensor_scalar(
                            out=htr[:], in0=ht[:],
                            scalar1=b1_sb[:, m:m + 1], scalar2=0.0,
                            op0=ALU.add, op1=ALU.max,
                        )
                    nc.tensor.matmul(
                        out_ps[m][32 * m:32 * m + C, :],
                        lhsT=w2_sb[:, m, :],
                        rhs=htr[:],
                        start=(g == 0),
                        stop=(g == bag - 1),
                        tile_position=(0, 32 * m),
                    )

            # DVE can read at most one PSUM operand per op: chain via SBUF
            acc = fpool.tile([C, bpc], F32, tag="acc")
            nc.vector.tensor_copy(acc[:], out_ps[0][0:C, :])
            for m in range(1, MT):
                nc.vector.tensor_add(
                    acc[:], acc[:], out_ps[m][32 * m:32 * m + C, :])
            fin = fpool.tile([C, bpc], F32, tag="fin")
            nc.scalar.add(fin[:], acc[:], add=b2_sb[:])
            nc.sync.dma_start(out=out[:], in_=fin[:])


def _np_cdt(cdt=None):
    if cdt is None:
        cdt = CDT
    if cdt == BF16:
        import ml_dtypes
        return ml_dtypes.bfloat16
    return np.float32


def host_prep_shared(W1, b1, W2, b2, bag=BAG_SIZE, cdt=None):
    np_cdt = _np_cdt(cdt)
    w1h = np.ascontiguousarray(
        W1.reshape(KT, 128, H).transpose(1, 0, 2)).astype(np_cdt)
    b1h = np.ascontiguousarray(
        b1.reshape(MT, 128).T).astype(np.float32)
    w2h = np.ascontiguousarray(
        (W2 / bag).reshape(MT, 128, C).transpose(1, 0, 2)).astype(np_cdt)
    b2h = np.ascontiguousarray(b2.reshape(C, 1)).astype(np.float32)
    return {"w1h": w1h, "b1h": b1h, "w2h": w2h, "b2h": b2h}


def host_prep_x(xs, bag=BAG_SIZE, cdt=None):
    np_cdt = _np_cdt(cdt)
    """xs: [r, D] rows of one core -> xh [128, KT, r] permuted."""
    r = xs.shape[0]
    bpc = r // bag
    xh = xs.reshape(bpc, bag, KT, 128).transpose(3, 2, 1, 0).reshape(128, KT, r)
    return np.ascontiguousarray(xh).astype(np_cdt)


_BUILT = None


def _get_built():
    global _BUILT
    if _BUILT is None:
        nc = bacc.Bacc("TRN2")
        build(nc)
        nc.compile()
        _BUILT = nc
    return _BUILT


def run(x, W1, b1, W2, b2, ids=None, trace=False):
    from concourse.bass_utils import run_bass_kernel_spmd

    nc = _get_built()
    shared = host_prep_shared(W1, b1, W2, b2)
    in_maps = []
    for c in range(N_CORES):
        xs = np.asarray(x[c * R:(c + 1) * R])
        in_maps.append({"xh": host_prep_x(xs), **shared})
    res = run_bass_kernel_spmd(
        nc, in_maps, core_ids=list(range(N_CORES)), trace=trace
    )
    outs = [res.results[c]["out"] for c in range(N_CORES)]
    full = np.concatenate([o.T for o in outs], axis=0).astype(np.float32)
    return full, res


def kernel(x, W1, b1, W2, b2, ids=None):
    full, _ = run(x, W1, b1, W2, b2, ids)
    return full


# revision 19
# speedup vs baseline: 4.7257x; 1.3432x over previous
"""BagModel (segment_reduce) Trainium2 kernel.

Computes out = (1/64 * segment_sum(relu(x @ W1 + b1))) @ W2 + b2 for
4096 bags of exactly 64 consecutive rows each, sharded bag-aligned
across 8 NeuronCores (512 bags / 32768 rows per core, weights
replicated, no cross-core communication).

Layout trick: the host permutes each core's x-shard to
    xh[p, k, g*512 + b] = x[b*64 + g, 128*k + p]
so row-group g contains row g of every bag, with the contraction dim D
on partitions.  The per-bag segment-sum then falls out of PSUM matmul
accumulation: the second (W2) matmul accumulates over the 64 row-groups
with start=(g==0)/stop=(g==63), so no explicit reduction pass over h is
ever needed.  The 4 H-slices of the W2 matmul go to 4 distinct PE
column-groups (tile_position) so they overlap in the array.
"""

import numpy as np

import concourse.bass as bass
import concourse.tile as tile
from concourse import bacc, mybir

N, D, H, C = 262144, 256, 512, 10
N_BAGS, BAG_SIZE = 4096, 64
N_CORES = 8
R = N // N_CORES            # rows per core
BPC = N_BAGS // N_CORES     # bags per core == free dim of each row-group
KT = D // 128               # contraction tiles (2)
MT = H // 128               # H tiles (4)

F32 = mybir.dt.float32
BF16 = mybir.dt.bfloat16
AF = mybir.ActivationFunctionType
ALU = mybir.AluOpType

# compute dtype for the matmul operands: bf16 is 4x faster on the PE
# (fp32 lowers to 2 LOW/HIGH passes at 2 cycles/elem); accumulation
# stays fp32 in PSUM either way
CDT = BF16


def build(nc: bass.Bass, bag: int = BAG_SIZE, bpc: int = BPC, cdt=None):
    """Emit the per-core program.  bag = rows per bag (= number of
    row-groups), bpc = bags per core (= free dim, <= 512)."""
    if cdt is None:
        cdt = CDT
    r = bag * bpc
    xT = nc.declare_dram_parameter("xh", [128, KT, r], cdt, isOutput=False)
    w1 = nc.declare_dram_parameter("w1h", [128, KT, H], cdt, isOutput=False)
    b1 = nc.declare_dram_parameter("b1h", [128, MT], F32, isOutput=False)
    w2 = nc.declare_dram_parameter("w2h", [128, MT, C], cdt, isOutput=False)
    b2 = nc.declare_dram_parameter("b2h", [C, 1], F32, isOutput=False)
    out = nc.declare_dram_parameter("out", [C, bpc], F32, isOutput=True)

    with tile.TileContext(nc) as tc:
        with (
            tc.tile_pool(name="const", bufs=1) as cpool,
            # bufs=8 matches the 8-queue HWDGE rotation: slot reuse then
            # pairs WAW deps on the same queue (implicit FIFO, no extra
            # sync wait — walrus allows only one non-self wait per inst)
            tc.tile_pool(name="xin", bufs=8) as xpool,
            tc.tile_pool(name="hrelu", bufs=3) as hpool,
            tc.tile_pool(name="fin", bufs=1) as fpool,
            tc.tile_pool(name="ps_ht", bufs=4, space="PSUM") as pspool,
            tc.tile_pool(name="ps_out", bufs=1, space="PSUM") as popool,
        ):
            w1_sb = cpool.tile([128, KT, H], cdt)
            nc.sync.dma_start(out=w1_sb[:], in_=w1[:])
            b1_sb = cpool.tile([128, MT], F32)
            nc.sync.dma_start(out=b1_sb[:], in_=b1[:])
            w2_sb = cpool.tile([128, MT, C], cdt)
            nc.sync.dma_start(out=w2_sb[:], in_=w2[:])
            b2_sb = cpool.tile([C, 1], F32)
            nc.sync.dma_start(out=b2_sb[:], in_=b2[:])

            # bag-sum accumulators: col-group m holds partial (over H slice
            # m) of out.T at partitions [32m, 32m+10) of its own bank —
            # start=True zeroes has_written bits bank-wide, so interleaved
            # accumulation groups must not share a bank
            out_ps = [popool.tile([128, bpc], F32, tag=f"out{m}",
                                  name=f"out_ps{m}")
                      for m in range(MT)]

            # warm-up matmuls: absorb the weight-DMA queue waits into PE's
            # vector clock so steady-state matmuls carry <=1 sync wait
            # (walrus limit); outputs are garbage, overwritten by the
            # start=True of the g==0 accumulation groups below.
            nc.tensor.matmul(
                out_ps[0][:, :], lhsT=w1_sb[:, 0, 0:128],
                rhs=w1_sb[:, 0, 0:bpc], start=True, stop=True,
            )
            nc.tensor.matmul(
                out_ps[1][0:C, :], lhsT=w2_sb[:, 0, :],
                rhs=w1_sb[:, 0, 0:bpc], start=True, stop=True,
            )
            # same for the bias tiles on ACT and DVE (first-touch waits)
            warm_a = cpool.tile([128, 1], F32, name="warm_a")
            nc.scalar.activation(warm_a[:], b1_sb[:, 0:1], AF.Relu,
                                 bias=b1_sb[:, 0:1], scale=1.0)
            warm_b = cpool.tile([C, 1], F32, name="warm_b")
            nc.scalar.activation(warm_b[:], b2_sb[:], AF.Relu,
                                 bias=b2_sb[:], scale=1.0)
            warm_v = cpool.tile([128, 1], F32, name="warm_v")
            nc.vector.tensor_scalar(
                out=warm_v[:], in0=b1_sb[:, 0:1], scalar1=b1_sb[:, 0:1],
                scalar2=0.0, op0=ALU.add, op1=ALU.max,
            )

            # software pipeline: W2 matmuls run one row-group behind the
            # W1 matmuls, so PE never stalls waiting for the ReLU results
            # (ReLU of group g overlaps the main matmuls of group g+1);
            # the 4 W2 matmuls are emitted back-to-back into 4 distinct PE
            # column groups so they overlap in the array.
            def emit_w2(gprev, htr_prev):
                for m in range(MT):
                    nc.tensor.matmul(
                        out_ps[m][32 * m:32 * m + C, :],
                        lhsT=w2_sb[:, m, :],
                        rhs=htr_prev[m][:],
                        start=(gprev == 0),
                        stop=(gprev == bag - 1),
                        tile_position=(0, 32 * m),
                    )

            prev = None
            for g in range(bag):
                xk = xpool.tile([128, KT, bpc], cdt, tag="xk")
                nc.sync.dma_start(
                    out=xk[:], in_=xT[:, :, g * bpc:(g + 1) * bpc]
                )
                htrs = []
                for m in range(MT):
                    ht = pspool.tile([128, bpc], F32, tag="ht")
                    for k in range(KT):
                        nc.tensor.matmul(
                            ht[:],
                            lhsT=w1_sb[:, k, 128 * m:128 * (m + 1)],
                            rhs=xk[:, k, :],
                            start=(k == 0),
                            stop=(k == KT - 1),
                        )
                    htr = hpool.tile([128, bpc], cdt, tag=f"htr{m}",
                                     name=f"htr_{g}_{m}")
                    if m < 2:
                        nc.scalar.activation(
                            htr[:], ht[:], AF.Relu,
                            bias=b1_sb[:, m:m + 1], scale=1.0,
                        )
                    else:
                        nc.vector.tensor_scalar(
                            out=htr[:], in0=ht[:],
                            scalar1=b1_sb[:, m:m + 1], scalar2=0.0,
                            op0=ALU.add, op1=ALU.max,
                        )
                    htrs.append(htr)
                if prev is not None:
                    emit_w2(g - 1, prev)
                prev = htrs
            emit_w2(bag - 1, prev)

            # DVE can read at most one PSUM operand per op: chain via SBUF
            acc = fpool.tile([C, bpc], F32, tag="acc")
            nc.vector.tensor_copy(acc[:], out_ps[0][0:C, :])
            for m in range(1, MT):
                nc.vector.tensor_add(
                    acc[:], acc[:], out_ps[m][32 * m:32 * m + C, :])
            fin = fpool.tile([C, bpc], F32, tag="fin")
            nc.scalar.add(fin[:], acc[:], add=b2_sb[:])
            nc.sync.dma_start(out=out[:], in_=fin[:])


def _np_cdt(cdt=None):
    if cdt is None:
        cdt = CDT
    if cdt == BF16:
        import ml_dtypes
        return ml_dtypes.bfloat16
    return np.float32


def host_prep_shared(W1, b1, W2, b2, bag=BAG_SIZE, cdt=None):
    np_cdt = _np_cdt(cdt)
    w1h = np.ascontiguousarray(
        W1.reshape(KT, 128, H).transpose(1, 0, 2)).astype(np_cdt)
    b1h = np.ascontiguousarray(
        b1.reshape(MT, 128).T).astype(np.float32)
    w2h = np.ascontiguousarray(
        (W2 / bag).reshape(MT, 128, C).transpose(1, 0, 2)).astype(np_cdt)
    b2h = np.ascontiguousarray(b2.reshape(C, 1)).astype(np.float32)
    return {"w1h": w1h, "b1h": b1h, "w2h": w2h, "b2h": b2h}


def host_prep_x(xs, bag=BAG_SIZE, cdt=None):
    np_cdt = _np_cdt(cdt)
    """xs: [r, D] rows of one core -> xh [128, KT, r] permuted."""
    r = xs.shape[0]
    bpc = r // bag
    xh = xs.reshape(bpc, bag, KT, 128).transpose(3, 2, 1, 0).reshape(128, KT, r)
    return np.ascontiguousarray(xh).astype(np_cdt)


_BUILT = None


def _get_built():
    global _BUILT
    if _BUILT is None:
        nc = bacc.Bacc("TRN2")
        build(nc)
        nc.compile()
        _BUILT = nc
    return _BUILT


def run(x, W1, b1, W2, b2, ids=None, trace=False):
    from concourse.bass_utils import run_bass_kernel_spmd

    nc = _get_built()
    shared = host_prep_shared(W1, b1, W2, b2)
    in_maps = []
    for c in range(N_CORES):
        xs = np.asarray(x[c * R:(c + 1) * R])
        in_maps.append({"xh": host_prep_x(xs), **shared})
    res = run_bass_kernel_spmd(
        nc, in_maps, core_ids=list(range(N_CORES)), trace=trace
    )
    outs = [res.results[c]["out"] for c in range(N_CORES)]
    full = np.concatenate([o.T for o in outs], axis=0).astype(np.float32)
    return full, res


def kernel(x, W1, b1, W2, b2, ids=None):
    full, _ = run(x, W1, b1, W2, b2, ids)
    return full
